# revision 1
# baseline (speedup 1.0000x reference)
"""SAGAN-style attention block on 8 TRN2 NeuronCores, data-parallel over batch.

Reference computation (per image, x: [64, 64, 512]):
    theta = x @ W_theta                     [4096, 64]
    phi   = maxpool2x2(x @ W_phi)           [1024, 64]
    g     = maxpool2x2(x @ W_g)             [1024, 256]
    beta  = softmax(theta @ phi.T, axis=-1) [4096, 1024]
    o     = (beta @ g) @ W_o                [4096, 512]
    out   = gamma * o + x

Sharding: batch 16 -> 2 images per core. No collectives.

Kernel layout strategy (per image):
  - x is transposed on the PE (via identity matmul) to xT [C=512 on
    partitions, S=4096 free] so projections can contract over C.
  - Projections are computed feature-major (d on partitions): thetaT [64, S],
    phiT [64, S] -> maxpool in free dim -> [64, 1024], gT [256, S] -> pool ->
    [256, 1024] -> PE-transpose to key-major g_aug [k, 257] with a ones column
    appended (column 256), so the attnV matmul also produces the softmax
    denominator for free.
  - scoresT [k, q] = phiT.T @ thetaT, softmax-exp applied by the ScalarE
    directly on the PSUM->SBUF evacuation (no max subtraction: |scores| < 60,
    safe in f32).
  - tmp [q, 257] = expT.T @ g_aug accumulated over k chunks; column 256 is the
    sum of exp. out = (tmp[:, :256] @ (gamma*W_o)) * (1/sum) + x, with the
    normalization folded into the fused scalar_tensor_tensor residual op.
  - All matmuls run in float32r (full-rate fp32, ~2e-4 rel err).
"""
import sys
import numpy as np

sys.path.insert(0, "/opt/trn_rl_repo")

from contextlib import ExitStack

import concourse.bass as bass
import concourse.tile as tile
from concourse import bacc, mybir
from concourse.bass_utils import run_bass_kernel_spmd

N_CORES = 8
IMG = 2            # images per core
H = W = 64
S = H * W          # 4096 queries per image
SK = S // 4        # 1024 keys after 2x2 maxpool
C = 512
D = C // 8         # 64
V = C // 2         # 256
QT = 512           # query tile
NQT = S // QT      # 8
QS = 128           # query subtile (partition dim)
NQS = QT // QS     # 4

F32 = mybir.dt.float32
F32R = mybir.dt.float32r
BF16 = mybir.dt.bfloat16

_CACHED_NC = None


def _build():
    nc = bacc.Bacc("TRN2", target_bir_lowering=False, debug=False,
                   num_devices=N_CORES)
    x_d = nc.dram_tensor("x", [IMG, S, C], F32, kind="ExternalInput").ap()
    wt_d = nc.dram_tensor("w_theta", [C, D], F32, kind="ExternalInput").ap()
    wp_d = nc.dram_tensor("w_phi", [C, D], F32, kind="ExternalInput").ap()
    wg_d = nc.dram_tensor("w_g", [C, V], F32, kind="ExternalInput").ap()
    wo_d = nc.dram_tensor("w_o", [V, C], F32, kind="ExternalInput").ap()
    id_d = nc.dram_tensor("ident", [128, 129], F32, kind="ExternalInput").ap()
    out_d = nc.dram_tensor("out", [IMG, S, C], F32, kind="ExternalOutput").ap()

    with tile.TileContext(nc) as tc, ExitStack() as ctx:
        # SBUF pools
        const_p = ctx.enter_context(tc.tile_pool(name="const", bufs=1))
        xin_p = ctx.enter_context(tc.tile_pool(name="xin", bufs=4))
        xt_p = ctx.enter_context(tc.tile_pool(name="xt", bufs=1))
        proj_p = ctx.enter_context(tc.tile_pool(name="proj", bufs=1))
        scr_p = ctx.enter_context(tc.tile_pool(name="scr", bufs=2))
        exp_p = ctx.enter_context(tc.tile_pool(name="exp", bufs=16))
        tmp_p = ctx.enter_context(tc.tile_pool(name="tmp", bufs=2))
        tt_p = ctx.enter_context(tc.tile_pool(name="tt", bufs=2))
        rc_p = ctx.enter_context(tc.tile_pool(name="rc", bufs=8))
        xr_p = ctx.enter_context(tc.tile_pool(name="xr", bufs=3))
        o_p = ctx.enter_context(tc.tile_pool(name="o", bufs=3))
        # PSUM pools
        psA = ctx.enter_context(tc.tile_pool(name="psA", bufs=2, space="PSUM"))
        psB = ctx.enter_context(tc.tile_pool(name="psB", bufs=2, space="PSUM"))
        psT = ctx.enter_context(tc.tile_pool(name="psT", bufs=2, space="PSUM"))
        psTT = ctx.enter_context(tc.tile_pool(name="psTT", bufs=2, space="PSUM"))

        # --- constants / weights (f32 staging -> f32r via DVE cast) ---
        ident_w = const_p.tile([128, 129], F32, tag="ident", name="ident_w")
        nc.sync.dma_start(ident_w[:], id_d[:])
        ident = ident_w[:, 0:128]
        ones_b = const_p.tile([128, 1], BF16, tag="ones_b", name="ones_b")
        nc.vector.tensor_copy(ones_b[:], ident_w[:, 128:129])
        ident_b = const_p.tile([128, 128], BF16, tag="ident_b", name="ident_b")
        nc.vector.tensor_copy(ident_b[:], ident_w[:, 0:128])

        wt_f = const_p.tile([128, 4 * D], F32, tag="wt_f", name="wt_f")
        wp_f = const_p.tile([128, 4 * D], F32, tag="wp_f", name="wp_f")
        wg_f = const_p.tile([128, 4 * V], F32, tag="wg_f", name="wg_f")
        wo_f = const_p.tile([128, 2 * C], F32, tag="wo_f", name="wo_f")
        for cc in range(4):
            nc.sync.dma_start(wt_f[:, cc * D:(cc + 1) * D],
                              wt_d[cc * 128:(cc + 1) * 128, :])
            nc.sync.dma_start(wp_f[:, cc * D:(cc + 1) * D],
                              wp_d[cc * 128:(cc + 1) * 128, :])
            nc.sync.dma_start(wg_f[:, cc * V:(cc + 1) * V],
                              wg_d[cc * 128:(cc + 1) * 128, :])
        for vc in range(2):
            nc.sync.dma_start(wo_f[:, vc * C:(vc + 1) * C],
                              wo_d[vc * 128:(vc + 1) * 128, :])
        wtp = const_p.tile([128, 4 * 2 * D], F32R, tag="wtp", name="wtp")
        wg = const_p.tile([128, 4 * V], F32R, tag="wg", name="wg")
        wo = const_p.tile([128, 2 * C], BF16, tag="wo", name="wo")
        for cc in range(4):
            nc.vector.tensor_copy(wtp[:, cc * 128:cc * 128 + D],
                                  wt_f[:, cc * D:(cc + 1) * D])
            nc.vector.tensor_copy(wtp[:, cc * 128 + D:(cc + 1) * 128],
                                  wp_f[:, cc * D:(cc + 1) * D])
        nc.vector.tensor_copy(wg[:], wg_f[:])
        nc.vector.tensor_copy(wo[:], wo_f[:])

        for img in range(IMG):
            # ---------- Phase A: load x, transpose to xT [4 x [128c, S]] ----
            xT = [xt_p.tile([128, S], F32R, tag=f"xT{cc}", name=f"xT{cc}") for cc in range(4)]
            for qb in range(NQT):  # blocks of 512 queries
                xin = [xin_p.tile([128, C], F32, tag="xin", name="xin") for _ in range(4)]
                for si in range(4):
                    q0 = qb * QT + si * 128
                    nc.sync.dma_start(xin[si][:], x_d[img, q0:q0 + 128, :])
                for cc in range(4):
                    tp = psT.tile([128, 512], F32, tag="tp", name="tp")
                    for si in range(4):
                        nc.tensor.transpose(tp[:, si * 128:(si + 1) * 128],
                                            xin[si][:, cc * 128:(cc + 1) * 128],
                                            ident[:])
                    nc.vector.tensor_copy(xT[cc][:, qb * QT:(qb + 1) * QT], tp[:])

            # ---------- Phase B: projections ----------
            thetaT = proj_p.tile([64, S], F32R, tag="thetaT", name="thetaT")
            phiT = proj_p.tile([64, SK], F32R, tag="phiT", name="phiT")
            gTp = [proj_p.tile([128, SK], F32R, tag=f"gTp{vc}", name=f"gTp{vc}") for vc in range(2)]
            m1 = scr_p.tile([64, 256], F32, tag="m1", name="m1")

            for qt in range(NQT):
                qsl = slice(qt * QT, (qt + 1) * QT)
                tp_ps = psB.tile([128, QT], F32, tag="mm", name="mm")
                for cc in range(4):
                    nc.tensor.matmul(tp_ps[:], wtp[:, cc * 128:(cc + 1) * 128],
                                     xT[cc][:, qsl], start=(cc == 0), stop=(cc == 3))
                nc.scalar.copy(thetaT[:, qsl], tp_ps[0:64, :])
                # Evacuate phi rows via ScalarE, then maxpool 2x2 in the free
                # dim: QT=512 covers 8 rows of w=64 -> [p, h2=4, hp=2, w2=32, wp=2]
                ph_sb = scr_p.tile([64, QT], F32, tag="ph_sb", name="ph_sb")
                nc.scalar.copy(ph_sb[:], tp_ps[64:128, :])
                pv = ph_sb.rearrange("p (a b c d) -> p a b c d", b=2, c=32, d=2)
                ma = m1[:, :128].rearrange("p (a c) -> p a c", a=4)
                mb = m1[:, 128:].rearrange("p (a c) -> p a c", a=4)
                po = phiT[:, qt * 128:(qt + 1) * 128].rearrange(
                    "p (a c) -> p a c", a=4)
                nc.vector.tensor_tensor(ma, pv[:, :, 0, :, 0], pv[:, :, 0, :, 1],
                                        mybir.AluOpType.max)
                nc.vector.tensor_tensor(mb, pv[:, :, 1, :, 0], pv[:, :, 1, :, 1],
                                        mybir.AluOpType.max)
                nc.vector.tensor_tensor(po, ma, mb, mybir.AluOpType.max)

                for vc in range(2):
                    g_ps = psB.tile([128, QT], F32, tag="mm", name="mm")
                    for cc in range(4):
                        nc.tensor.matmul(
                            g_ps[:], wg[:, cc * V + vc * 128: cc * V + (vc + 1) * 128],
                            xT[cc][:, qsl], start=(cc == 0), stop=(cc == 3))
                    g_sb = scr_p.tile([128, QT], F32, tag="g_sb", name="g_sb")
                    nc.scalar.copy(g_sb[:], g_ps[:])
                    gv = g_sb.rearrange("p (a b c d) -> p a b c d", b=2, c=32, d=2)
                    m2 = scr_p.tile([128, 256], F32, tag="m2", name="m2")
                    ga = m2[:, :128].rearrange("p (a c) -> p a c", a=4)
                    gb = m2[:, 128:].rearrange("p (a c) -> p a c", a=4)
                    go = gTp[vc][:, qt * 128:(qt + 1) * 128].rearrange(
                        "p (a c) -> p a c", a=4)
                    nc.vector.tensor_tensor(ga, gv[:, :, 0, :, 0], gv[:, :, 0, :, 1],
                                            mybir.AluOpType.max)
                    nc.vector.tensor_tensor(gb, gv[:, :, 1, :, 0], gv[:, :, 1, :, 1],
                                            mybir.AluOpType.max)
                    nc.vector.tensor_tensor(go, ga, gb, mybir.AluOpType.max)

            # g -> key-major with ones column: g_aug[kc] = [128k, 257]
            g_aug = []
            for kc in range(8):
                ga_t = proj_p.tile([128, V + 2], BF16, tag=f"gaug{kc}", name=f"gaug{kc}")
                tp = psT.tile([128, 256], F32, tag="tp", name="tp")
                for vc in range(2):
                    nc.tensor.transpose(
                        tp[:, vc * 128:(vc + 1) * 128],
                        gTp[vc][:, kc * 128:(kc + 1) * 128].bitcast(F32), ident[:])
                nc.vector.tensor_copy(ga_t[:, 0:V], tp[:])
                nc.vector.tensor_copy(ga_t[:, V:V + 1], ones_b[:])
                nc.vector.tensor_copy(ga_t[:, V + 1:V + 2], ones_b[:])
                g_aug.append(ga_t)

            # ---------- Phase C: attention per q-tile ----------
            for qt in range(NQT):
                qsl = slice(qt * QT, (qt + 1) * QT)
                # scoresT [k, q] + exp
                ex = []
                for kc in range(8):
                    sc_ps = psA.tile([128, QT], F32, tag="sc", name="sc")
                    nc.tensor.matmul(sc_ps[:], phiT[:, kc * 128:(kc + 1) * 128],
                                     thetaT[:, qsl], start=True, stop=True)
                    e_t = exp_p.tile([128, QT], BF16, tag="exp", name="exp")
                    nc.scalar.activation(e_t[:], sc_ps[:],
                                         mybir.ActivationFunctionType.Exp)
                    ex.append(e_t)
                # attnV, transposed route: tmpT[v, q] = g_aug.T @ expT, the
                # M=2 ones-rows slice also yields the softmax sums [2, q].
                tv_ps = [psTT.tile([128, QT], F32, tag="tv", name="tv")
                         for _ in range(2)]
                sum_ps = psB.tile([2, QT], F32, tag="mm", name="mm")
                for kc in range(8):
                    st = (kc == 0)
                    sp = (kc == 7)
                    for vc in range(2):
                        nc.tensor.matmul(tv_ps[vc][:],
                                         g_aug[kc][:, vc * 128:(vc + 1) * 128],
                                         ex[kc][:], start=st, stop=sp)
                    nc.tensor.matmul(sum_ps[:], g_aug[kc][:, V:V + 2], ex[kc][:],
                                     start=st, stop=sp)
                # reciprocal of sums, per q-subtile partition-major:
                # copy the [2, QT] sum row to SBUF, PE-transpose 128-wide
                # chunks to [128, 2], then DVE reciprocal.
                srow = scr_p.tile([2, QT], F32, tag="srow", name="srow")
                nc.vector.tensor_copy(srow[:], sum_ps[:])
                rc_ps = psT.tile([128, 2 * NQS], F32, tag="tp", name="tp")
                recs = []
                for qs in range(NQS):
                    nc.tensor.transpose(rc_ps[:, qs * 2:qs * 2 + 2],
                                        srow[:, qs * 128:(qs + 1) * 128],
                                        ident[0:2, 0:2])
                for qs in range(NQS):
                    rc = rc_p.tile([128, 1], F32, tag="rc", name="rc")
                    nc.vector.reciprocal(rc[:], rc_ps[:, qs * 2:qs * 2 + 1])
                    recs.append(rc)
                tT = [tt_p.tile([128, QT], BF16, tag=f"tt{vc}", name=f"tt{vc}") for vc in range(2)]
                for vc in range(2):
                    nc.vector.tensor_copy(tT[vc][:], tv_ps[vc][:])
                for qs in range(NQS):
                    ssl = slice(qs * 128, (qs + 1) * 128)
                    o_ps = psB.tile([128, C], F32, tag="mm", name="mm")
                    for vc in range(2):
                        nc.tensor.matmul(o_ps[:], tT[vc][:, ssl],
                                         wo[:, vc * C:(vc + 1) * C],
                                         start=(vc == 0), stop=(vc == 1))
                    q0 = qt * QT + qs * 128
                    xr = xr_p.tile([128, C], F32, tag="xr", name="xr")
                    nc.sync.dma_start(xr[:], x_d[img, q0:q0 + 128, :])
                    ot = o_p.tile([128, C], F32, tag="ot", name="ot")
                    # out = (o * (1/sum)) + x   (gamma pre-folded into W_o)
                    nc.vector.scalar_tensor_tensor(
                        ot[:], o_ps[:], recs[qs][:], xr[:],
                        mybir.AluOpType.mult, mybir.AluOpType.add)
                    nc.sync.dma_start(out_d[img, q0:q0 + 128, :], ot[:])

    nc.compile()
    return nc


def _get_nc():
    global _CACHED_NC
    if _CACHED_NC is None:
        _CACHED_NC = _build()
    return _CACHED_NC


def _run(inputs, trace=False, trace_kwargs=None):
    x = np.ascontiguousarray(np.asarray(inputs["x"], dtype=np.float32))
    wt = np.ascontiguousarray(np.asarray(inputs["W_theta"], dtype=np.float32))
    wp = np.ascontiguousarray(np.asarray(inputs["W_phi"], dtype=np.float32))
    wg = np.ascontiguousarray(np.asarray(inputs["W_g"], dtype=np.float32))
    wo = np.ascontiguousarray(np.asarray(inputs["W_o"], dtype=np.float32))
    gamma = np.float64(np.asarray(inputs["gamma"], dtype=np.float32))
    wo_s = np.ascontiguousarray((gamma * wo.astype(np.float64)).astype(np.float32))
    ident = np.concatenate([np.eye(128, dtype=np.float32),
                            np.ones((128, 1), dtype=np.float32)], axis=1)

    B = x.shape[0]
    assert B == N_CORES * IMG
    xs = x.reshape(B, S, C)
    in_maps = []
    for i in range(N_CORES):
        in_maps.append({
            "x": np.ascontiguousarray(xs[i * IMG:(i + 1) * IMG]),
            "w_theta": wt, "w_phi": wp, "w_g": wg, "w_o": wo_s,
            "ident": ident,
        })
    nc = _get_nc()
    kw = {}
    if trace:
        kw["trace"] = True
        if trace_kwargs:
            kw["trace_kwargs"] = trace_kwargs
    res = run_bass_kernel_spmd(nc, in_maps, core_ids=list(range(N_CORES)), **kw)
    outs = [res.results[i]["out"].reshape(IMG, H, W, C) for i in range(N_CORES)]
    full = np.concatenate(outs, axis=0)
    return full, res


def kernel(**inputs):
    full, _ = _run(inputs, trace=False)
    return full

